# revision 92
# baseline (speedup 1.0000x reference)
"""MoE-Mamba-Transformer block on 8 TRN2 cores (SPMD, no collectives).

Sharding: core c -> (batch b = c//2, sequence-half h = c%2); each core
computes the output for its 512 "own" tokens given full-sequence context,
host gathers. Device is feature-major ([feature, token]); host transposes
x/memory/output, folds LayerNorm affine params into downstream weights,
and pre-rounds all GEMM weights to bf16 (PE runs 1 cycle/row vs 4 for
fp32; bf16 weights also re-enable fast weight load).

Schedule: router + self-attention projections are emitted before the
Mamba section, and self-attention head i is emitted after Mamba chunk i
inside the 8-chunk SSD loop, so PE/ACT fill the DVE-bound chunk loop's
gaps (engine queues are FIFO per engine; program order = issue order).
Cross-attention K/V (memory-only deps) fill the post-loop gap. Softmax
prefix masking is folded into a per-core denominator-correction bias;
only diagonal key tiles carry an additive causal mask. Norm/softmax
reciprocals use the single-instruction DVE approx (SBUF-staged - the
approx misreads PSUM operands). The Mamba scan uses the chunked SSD
formulation (matmuls + one cumsum in fp32; per-head small matmuls in
bf16 with fp32 PSUM accumulation; bf16 inter-chunk state).
"""

import numpy as np
from contextlib import ExitStack

import concourse.bass as bass
import concourse.mybir as mybir
import concourse.tile as tile
from concourse import bacc, bass_utils
from concourse.masks import make_identity

F32 = mybir.dt.float32
F32R = mybir.dt.float32r
BF16 = mybir.dt.bfloat16
AOP = mybir.AluOpType
AF = mybir.ActivationFunctionType

D = 512
NHEAD = 8
HD = 64
DIN = 1024
HM = 16
P = 64
NST = 64
DCONV = 4
CONV = 1152
DPROJ = 2192
DFF = 2048
EPS = 1e-5
TO = 512
TT = 1024
LC = 128
NCH = TT // LC
KT = D // 128
B_, T_, S_ = 4, 1024, 1024

_CACHE = {}


def _mm(nc, out, lhsT, rhs, start, stop):
    nc.tensor.matmul(out, lhsT, rhs, start=start, stop=stop)


def _mmr(nc, out, lhsT, rhs, start, stop):
    nc.tensor.matmul(out, lhsT.bitcast(F32R), rhs.bitcast(F32R), start=start, stop=stop)


def build_program(dbg=False):
    nc = bacc.Bacc("TRN2", target_bir_lowering=False, debug=False, num_devices=8)
    din = lambda name, shape, dt=F32: nc.dram_tensor(name, shape, dt, kind="ExternalInput").ap()
    _dbg_outs = {}

    def dout(name, shape, dt=F32):
        _dbg_outs[name] = nc.dram_tensor("dbg_" + name, shape, dt, kind="ExternalOutput").ap()
        return _dbg_outs[name]

    xT = din("xT", [D, TT])
    memT = din("memT", [D, TT], BF16)
    maskT = din("maskT", [TT, TO], BF16)
    gmask = din("gmask", [LC, LC])
    selm = din("selm", [HM, 8 * 128])
    cscale = din("cscale", [128, 1])
    denc = din("denc", [1, 1])
    w_inproj = din("w_inproj", [D, DPROJ], BF16)
    b_inproj = din("b_inproj", [128, 18])
    conv_w = din("conv_w", [128, 36])
    conv_b = din("conv_b", [128, 9])
    dt_bias = din("dt_bias", [HM, 1])
    negA = din("negA", [HM, 1])
    dskip = din("dskip", [128, 8])
    w_mout = din("w_mout", [DIN, D], BF16)
    w_qkv = din("w_qkv", [D, 3 * D], BF16)
    b_qkv = din("b_qkv", [128, 12])
    vbias_row = din("vbias_row", [1, D])
    w_router = din("w_router", [D, 2], BF16)
    b_router = din("b_router", [2, 1])
    w_ao = din("w_ao", [D, D], BF16)
    b_ao = din("b_ao", [128, 4])
    w_q = din("w_q", [D, D], BF16)
    b_q = din("b_q", [128, 4])
    w_kv = din("w_kv", [D, 2 * D], BF16)
    w_co = din("w_co", [D, D], BF16)
    b_co = din("b_co", [128, 4])
    w_f1 = din("w_f1", [D, DFF], BF16)
    b_f1 = din("b_f1", [128, 16])
    w_f2 = din("w_f2", [DFF, D], BF16)
    b_f2 = din("b_f2", [128, 4])
    yout = nc.dram_tensor("y", [D, TO], F32, kind="ExternalOutput").ap()
    if dbg:
        for nm, sh in [("dtr", [HM, TT]), ("crow", [HM, TT]), ("om0", [128, TO]),
                       ("x10", [128, TO]), ("x20", [128, TO])]:
            dout(nm, sh)
        for nm, sh in [("zs0b", [128, TO]), ("xb0b", [128, TT]), ("xb8b", [128, TT]),
                       ("nx", [128, TT]), ("ymam0", [128, TO]), ("ymam0n", [128, TO]),
                       ("qf0", [128, TO]), ("kf0", [128, TT]), ("oattn0", [128, TO]),
                       ("n20", [128, TO]), ("qc0", [128, TO]), ("kc0", [128, TT]),
                       ("ocat0", [128, TO]), ("n30", [128, TO]), ("f10", [128, TO])]:
            dout(nm, sh, BF16)

    es = ExitStack()
    with es:
        tc = es.enter_context(tile.TileContext(nc))
        persist = es.enter_context(tc.tile_pool(name="persist", bufs=1))
        pbig = es.enter_context(tc.tile_pool(name="pbig", bufs=2, space="PSUM"))
        pt = es.enter_context(tc.tile_pool(name="pt", bufs=2, space="PSUM"))
        psu = es.enter_context(tc.tile_pool(name="psu", bufs=1, space="PSUM"))
        pstat = es.enter_context(tc.tile_pool(name="pstat", bufs=1, space="PSUM"))
        scratch = es.enter_context(tc.tile_pool(name="scratch", bufs=1))
        rows = es.enter_context(tc.tile_pool(name="rows", bufs=1))
        mampool = es.enter_context(tc.tile_pool(name="mampool", bufs=2))
        mbig = es.enter_context(tc.tile_pool(name="mbig", bufs=1))

        ident = persist.tile([128, 128], F32, tag="ident", name="ident")
        make_identity(nc, ident[:])
        identb = persist.tile([128, 128], BF16, tag="identb", name="identb")
        make_identity(nc, identb[:])
        onescol = persist.tile([128, 1], F32, tag="onescol", name="onescol")
        nc.vector.memset(onescol[:], 1.0)
        onescol_r = persist.tile([128, 1], F32R, tag="onescolr", name="onescolr")
        nc.vector.tensor_copy(onescol_r[:], onescol[:])
        epscol = persist.tile([128, 1], F32, tag="epscol", name="epscol")
        nc.vector.memset(epscol[:], EPS)

        def load(pool, ap, shape, tag, dtype=F32):
            t = pool.tile(shape, dtype, tag=tag)
            if dtype == F32R:
                nc.gpsimd.dma_start(t[:], ap)
            else:
                nc.sync.dma_start(t[:], ap)
            return t

        cscale_t = load(persist, cscale[:], [128, 1], "cscale")
        denc_t = load(persist, denc[:], [1, 1], "denc")
        dtb_t = load(persist, dt_bias[:], [HM, 1], "dtb")
        negA_t = load(persist, negA[:], [HM, 1], "negA")
        dskip_t = load(persist, dskip[:], [128, 8], "dskip")
        gmask_t = load(persist, gmask[:], [LC, LC], "gmask")
        binp_t = load(persist, b_inproj[:], [128, 18], "binp")
        cw_t = load(persist, conv_w[:], [128, 36], "cw")
        cb_t = load(persist, conv_b[:], [128, 9], "cb")
        bqkv_t = load(persist, b_qkv[:], [128, 12], "bqkv")
        bao_t = load(persist, b_ao[:], [128, 4], "bao")
        bq_t = load(persist, b_q[:], [128, 4], "bq")
        bco_t = load(persist, b_co[:], [128, 4], "bco")
        bf1_t = load(persist, b_f1[:], [128, 16], "bf1")
        bf2_t = load(persist, b_f2[:], [128, 4], "bf2")
        brt_t = load(persist, b_router[:], [2, 1], "brt")
        vbr_t = load(persist, vbias_row[:], [1, D], "vbr")


        def ddump(name, ap, bf=False):
            if not dbg:
                return
            if ap.dtype == F32R:
                ap = ap.bitcast(F32)
            nc.sync.dma_start(_dbg_outs[name], ap)

        mxp = es.enter_context(tc.tile_pool(name="mixpool", bufs=1))
        ppool = es.enter_context(tc.tile_pool(name="ppool", bufs=9))
        attnpool = tc.tile_pool(name="attnp", bufs=1)
        ap_ = attnpool.__enter__()
        mamA = tc.tile_pool(name="mamA", bufs=1)
        mspA = mamA.__enter__()
        nxpool = tc.tile_pool(name="nxpool", bufs=1)
        nxp = nxpool.__enter__()
        xtpool = tc.tile_pool(name="xtpool", bufs=1)
        xtp = xtpool.__enter__()
        xt = [load(xtp, xT[k * 128:(k + 1) * 128, :], [128, TT], f"xt{k}") for k in range(KT)]

        # ---------- feature-major LayerNorm ----------
        def layernorm(xtiles, TL, nfeat, outpool, tag, odt=F32):
            nk = len(xtiles)
            bca = scratch.tile([128, TL], F32, tag="lnbc", name="lnbca", bufs=2)
            bcb = scratch.tile([128, TL], F32, tag="lnbc", name="lnbcb", bufs=2)
            onescol_b = scratch.tile([128, 1], BF16, tag="onescolb", name="onescolb", bufs=1)
            nc.vector.tensor_copy(onescol_b[:], onescol[:])
            for g in range(TL // 512):
                sl = slice(g * 512, (g + 1) * 512)
                sum_ps = pstat.tile([1, 512], F32, tag="statA", name="statA")
                sq_ps = pstat.tile([1, 512], F32, tag="statB", name="statB")
                for k in range(nk):
                    xb = scratch.tile([128, 512], BF16, tag="lnxb", name="lnxb", bufs=2)
                    nc.vector.tensor_copy(xb[:], xtiles[k][:, sl])
                    _mm(nc, sum_ps[:], onescol_b[:], xb[:], k == 0, k == nk - 1)
                    sqt = scratch.tile([128, 512], BF16, tag="lnsq", name="lnsq", bufs=2)
                    nc.scalar.activation(sqt[:], xb[:], AF.Square)
                    _mm(nc, sq_ps[:], onescol_b[:], sqt[:], k == 0, k == nk - 1)
                r_m2 = rows.tile([1, 512], F32, tag="rsm", name="r1", bufs=3)
                nc.scalar.activation(r_m2[:], sum_ps[:], AF.Square, scale=1.0 / nfeat)
                r_v = rows.tile([1, 512], F32, tag="rsm", name="r2", bufs=3)
                nc.vector.scalar_tensor_tensor(r_v[:], sq_ps[:], 1.0 / nfeat, r_m2[:], AOP.mult, AOP.subtract)
                r_s = rows.tile([1, 512], F32, tag="rsm", name="r3", bufs=3)
                nc.scalar.activation(r_s[:], r_v[:], AF.Sqrt, bias=epscol[0:1, :])
                ra = rows.tile([1, 512], F32, tag="rsm", name="ra", bufs=3)
                nc.vector.reciprocal_approx_fast(ra[:], r_s[:])
                r_mean = rows.tile([1, 512], F32, tag="rsm", name="r5", bufs=3)
                nc.scalar.activation(r_mean[:], sum_ps[:], AF.Copy, scale=1.0 / nfeat)
                rb = rows.tile([1, 512], F32, tag="rsm", name="rb", bufs=3)
                nc.vector.scalar_tensor_tensor(rb[:], r_mean[:], -1.0, ra[:], AOP.mult, AOP.mult)
                nc.gpsimd.partition_broadcast(bca[:, sl], ra[:])
                nc.gpsimd.partition_broadcast(bcb[:, sl], rb[:])
            out = []
            for k in range(nk):
                if odt != F32:
                    o = scratch.tile([128, TL], F32, tag="lnstage", name="lnstage", bufs=1)
                    orr = outpool.tile([128, TL], odt, tag=f"{tag}{k}", name=f"{tag}{k}")
                    for g in range(TL // 512):
                        sl2 = slice(g * 512, (g + 1) * 512)
                        nc.vector.tensor_mul(o[:, sl2], xtiles[k][:, sl2], bca[:, sl2])
                        nc.vector.tensor_add(orr[:, sl2], o[:, sl2], bcb[:, sl2])
                    o = orr
                else:
                    o = outpool.tile([128, TL], F32, tag=f"{tag}{k}", name=f"{tag}{k}")
                    nc.vector.tensor_mul(o[:], xtiles[k][:], bca[:])
                    nc.vector.tensor_add(o[:], o[:], bcb[:])
                out.append(o)
            return out

        nx = layernorm(xt, TT, D, nxp, "nx", odt=BF16)
        ddump("nx", nx[0][:])
        xtpool.__exit__(None, None, None)

        # ---------- router (early; needs only nx) ----------
        rps = pstat.tile([2, TO], F32, tag="statA", name="statB")
        with tc.tile_pool(name="rw", bufs=1) as rwp:
            wr = [load(rwp, w_router[k * 128:(k + 1) * 128, :], [128, 2], f"wr{k}", dtype=BF16) for k in range(KT)]
            for k in range(KT):
                _mm(nc, rps[:], wr[k][:], nx[k][:, TO:TT], k == 0, k == KT - 1)
            er = rows.tile([2, TO], F32R, tag="rer", name="er", bufs=1)
            nc.scalar.activation(er[:], rps[:], AF.Exp, bias=brt_t[:])
            dsum_ps = pstat.tile([1, TO], F32, tag="statB", name="dsumps")
            _mm(nc, dsum_ps[:], onescol_r[0:2, :], er[:], True, True)
            den_sb = rows.tile([1, TO], F32, tag="rsm", name="densb3", bufs=3)
            nc.scalar.activation(den_sb[:], dsum_ps[:], AF.Copy)
            drec = rows.tile([1, TO], F32, tag="rsm", name="drec", bufs=3)
            nc.vector.reciprocal_approx_fast(drec[:], den_sb[:])
            rw0r = rows.tile([1, TO], F32, tag="rsm", name="rw0r", bufs=3)
            nc.vector.tensor_mul(rw0r[:], er[0:1, :], drec[:])
            rw1r = rows.tile([1, TO], F32, tag="rsm", name="rw1r", bufs=3)
            nc.vector.tensor_scalar(out=rw1r[:], in0=rw0r[:], scalar1=-1.0, scalar2=1.0, op0=AOP.mult, op1=AOP.add)
            rw0b_r = rows.tile([1, TO], BF16, tag="rsmb", name="rw0br", bufs=2)
            nc.vector.tensor_copy(rw0b_r[:], rw0r[:])
            rw1b_r = rows.tile([1, TO], BF16, tag="rsmb", name="rw1br", bufs=2)
            nc.vector.tensor_copy(rw1b_r[:], rw1r[:])
            rw0bc = mxp.tile([128, TO], BF16, tag="rw0bc", name="rw0bc")
            rw1bc = mxp.tile([128, TO], BF16, tag="rw1bc", name="rw1bc")
            nc.gpsimd.partition_broadcast(rw0bc[:], rw0b_r[:])
            nc.gpsimd.partition_broadcast(rw1bc[:], rw1b_r[:])

        # ---------- attention projections (early; need only nx) ----------
        oattn = [mxp.tile([128, TO], BF16, tag=f"oattn{m}", name=f"oattn{m}") for m in range(KT)]
        with tc.tile_pool(name="aw", bufs=1) as aw:
            wq = [load(aw, w_qkv[k * 128:(k + 1) * 128, :], [128, 3 * D], f"wqkv{k}", dtype=BF16) for k in range(KT)]
            qf = [ap_.tile([128, TO], BF16, tag=f"qf{m}", name=f"qf{m}") for m in range(KT)]
            for m in range(KT):
                ps = pbig.tile([128, 512], F32, tag="mmain", name="mmain")
                for k in range(KT):
                    _mm(nc, ps[:], wq[k][:, m * 128:(m + 1) * 128], nx[k][:, TO:TT], k == 0, k == KT - 1)
                nc.scalar.activation(qf[m][:], ps[:], AF.Identity, bias=bqkv_t[:, m:m + 1])
            kf = [ap_.tile([128, TT], BF16, tag=f"kf{m}", name=f"kf{m}") for m in range(KT)]
            for m in range(KT):
                for g in range(2):
                    ps = pbig.tile([128, 512], F32, tag="mmain", name="mmain")
                    for k in range(KT):
                        _mm(nc, ps[:], wq[k][:, D + m * 128:D + (m + 1) * 128],
                            nx[k][:, g * 512:(g + 1) * 512], k == 0, k == KT - 1)
                    nc.scalar.activation(kf[m][:, g * 512:(g + 1) * 512], ps[:], AF.Identity,
                                         bias=bqkv_t[:, 4 + m:5 + m])
            ddump("qf0", qf[0][:])
            ddump("kf0", kf[0][:])
            vext = [ap_.tile([128, NHEAD * (HD + 1)], BF16, tag=f"vext{s}", name=f"vext{s}") for s in range(8)]
            vb_bc = aw.tile([128, D], F32, tag="vbbc", name="vbbc")
            nc.gpsimd.partition_broadcast(vb_bc[:], vbr_t[:])
            for s in range(8):
                ps = pbig.tile([128, 512], F32, tag="mmain", name="mmain")
                for k in range(KT):
                    _mm(nc, ps[:], nx[k][:, s * 128:(s + 1) * 128], wq[k][:, 2 * D:3 * D], k == 0, k == KT - 1)
                v3 = lambda apx: apx.rearrange("p (h d) -> p h d", h=NHEAD)
                nc.vector.tensor_add(vext[s][:].rearrange("p (h d) -> p h d", h=NHEAD)[:, :, 0:HD],
                                     v3(ps[:]), v3(vb_bc[:]))
                nc.vector.memset(vext[s][:].rearrange("p (h d) -> p h d", h=NHEAD)[:, :, HD:HD + 1], 1.0)

        def attn_head(hh):
            jj, half = hh // 2, hh % 2
            rsl = slice(half * HD, (half + 1) * HD)
            pT = []
            for s in range(8):
                ps = pbig.tile([128, 512], F32, tag="mmain", name="mmain")
                _mm(nc, ps[:], kf[jj][rsl, s * 128:(s + 1) * 128], qf[jj][rsl, :], True, True)
                pe = ppool.tile([128, TO], BF16, tag="pT", name="pT")
                if s < 4:
                    nc.scalar.activation(pe[:], ps[:], AF.Exp)
                else:
                    nc.vector.tensor_add(pe[:], ps[:], mkt[s - 4][:])
                    nc.scalar.activation(pe[:], pe[:], AF.Exp)
                pT.append(pe)
            ov_ps = pbig.tile([128, 512], F32, tag="mmain", name="mmain")
            for s in range(8):
                nc.tensor.matmul(ov_ps[0:HD + 1, :], vext[s][:, hh * (HD + 1):(hh + 1) * (HD + 1)],
                                 pT[s][:], start=(s == 0), stop=(s == 7))
            den_sb = rows.tile([1, TO], F32, tag="rsm", name="densb", bufs=3)
            nc.scalar.activation(den_sb[:], ov_ps[HD:HD + 1, :], AF.Identity, bias=denc_t[0:1, :])
            drow = rows.tile([1, TO], F32, tag="rsm", name="drow", bufs=3)
            nc.vector.reciprocal_approx_fast(drow[:], den_sb[:])
            dbc = scratch.tile([128, TO], F32, tag="bc512", name="dbc", bufs=2)
            nc.gpsimd.partition_broadcast(dbc[:], drow[:])
            nc.vector.tensor_mul(oattn[jj][rsl, :], ov_ps[0:HD, :], dbc[rsl, :])

        # ---------- mamba: in_proj + depthwise conv + dt ----------
        xbcc = [mspA.tile([128, TT], BF16, tag=f"xbcc{m}", name=f"xbcc{m}") for m in range(9)]
        zs = [mspA.tile([128, TO], BF16, tag=f"zs{m}", name=f"zs{m}") for m in range(8)]
        dtr = mspA.tile([HM, TT], F32, tag="dtr", name="dtr")

        with tc.tile_pool(name="mwz", bufs=1) as mwz:
            wdt = [load(mwz, w_inproj[k * 128:(k + 1) * 128, DPROJ - HM:DPROJ], [128, HM], f"wdt{k}", dtype=BF16) for k in range(KT)]
            for g in range(2):
                ps = pstat.tile([HM, 512], F32, tag="statA", name="dtps")
                for k in range(KT):
                    _mm(nc, ps[:], wdt[k][:], nx[k][:, g * 512:(g + 1) * 512], k == 0, k == KT - 1)
                dsl = dtr[:, g * 512:(g + 1) * 512]
                nc.scalar.activation(dsl, ps[:], AF.Sigmoid, bias=dtb_t[:])
                nc.scalar.activation(dsl, dsl, AF.Ln)
                nc.vector.scalar_tensor_tensor(dsl, ps[:], dtb_t[:], dsl, AOP.add, AOP.subtract)
            for mh in range(2):
                wz = [load(mwz, w_inproj[k * 128:(k + 1) * 128, mh * 512:(mh + 1) * 512],
                           [128, 512], f"wz{k}", dtype=BF16) for k in range(KT)]
                for mm_ in range(4):
                    m = mh * 4 + mm_
                    ps = pbig.tile([128, 512], F32, tag="mmain", name="zps")
                    for k in range(KT):
                        _mm(nc, ps[:], wz[k][:, mm_ * 128:(mm_ + 1) * 128], nx[k][:, TO:TT], k == 0, k == KT - 1)
                    nc.scalar.activation(zs[m][:], ps[:], AF.Silu, bias=binp_t[:, m:m + 1])

        # ---------- mamba: cumsum + chunk decays ----------
        logdA = mspA.tile([HM, TT], F32, tag="logdA", name="logdA")
        nc.vector.tensor_scalar(out=logdA[:], in0=dtr[:], scalar1=negA_t[:], scalar2=None, op0=AOP.mult)
        zrow = mspA.tile([HM, TT], F32, tag="zrow", name="zrow")
        nc.vector.memset(zrow[:], 0.0)
        crow = mspA.tile([HM, TT], F32, tag="crow", name="crow")
        nc.vector.tensor_tensor_scan(crow[:], logdA[:], zrow[:], 0.0, AOP.add, AOP.add)

        negcte = mspA.tile([HM, NCH], F32, tag="negcte", name="negcte")
        dchunk_all = mspA.tile([HM, NCH], F32, tag="dchunk", name="dchunk")
        for i in range(NCH):
            te = (i + 1) * LC - 1
            nc.vector.tensor_scalar(out=negcte[:, i:i + 1], in0=crow[:, te:te + 1],
                                    scalar1=-1.0, scalar2=None, op0=AOP.mult)
        for i in range(NCH):
            te = (i + 1) * LC - 1
            if i == 0:
                nc.scalar.activation(dchunk_all[:, 0:1], crow[:, te:te + 1], AF.Exp)
            else:
                nc.scalar.activation(dchunk_all[:, i:i + 1], crow[:, te:te + 1], AF.Exp,
                                     bias=negcte[:, i - 1:i])

        ddump("crow", crow[:])
        selm_t = load(mspA, selm[:], [HM, 8 * 128], "selm")
        dchx16 = []
        for j in range(8):
            pp = pstat.tile([128, P], F32, tag="statB", name="psu")
            _mm(nc, pp[:, 0:8], selm_t[:, j * 128:(j + 1) * 128], dchunk_all[:], True, True)
            for half in range(2):
                sb = mspA.tile([P, 8], F32, tag=f"dchx{2 * j + half}", name=f"dchx{2 * j + half}")
                nc.vector.tensor_copy(sb[:], pp[half * P:(half + 1) * P, 0:8])
                dchx16.append(sb)

        with tc.tile_pool(name="mwx", bufs=1) as mwx, tc.tile_pool(name="rawp", bufs=2) as rawp:
            for wh in range(3):
                wx = [load(mwx, w_inproj[k * 128:(k + 1) * 128, DIN + wh * 384:DIN + (wh + 1) * 384],
                           [128, 384], f"wx{k}", dtype=BF16) for k in range(KT)]
                for mm_ in range(3):
                    m = wh * 3 + mm_
                    raw = rawp.tile([128, DCONV - 1 + TT], BF16, tag="raw", name="raw", bufs=2)
                    nc.vector.memset(raw[:, 0:DCONV - 1], 0.0)
                    for g in range(2):
                        ps = pbig.tile([128, 512], F32, tag="mmain", name="xps")
                        for k in range(KT):
                            _mm(nc, ps[:], wx[k][:, mm_ * 128:(mm_ + 1) * 128],
                                nx[k][:, g * 512:(g + 1) * 512], k == 0, k == KT - 1)
                        nc.scalar.activation(raw[:, 3 + g * 512:3 + (g + 1) * 512], ps[:],
                                             AF.Identity, bias=binp_t[:, 8 + m:9 + m])
                    acc = xbcc[m]
                    nc.vector.tensor_scalar(out=acc[:], in0=raw[:, 0:TT],
                                            scalar1=cw_t[:, 4 * m:4 * m + 1], scalar2=None, op0=AOP.mult)
                    for kk in range(1, DCONV):
                        nc.vector.scalar_tensor_tensor(acc[:], raw[:, kk:kk + TT],
                                                       cw_t[:, 4 * m + kk:4 * m + kk + 1], acc[:],
                                                       AOP.mult, AOP.add)
                    nc.scalar.activation(xbcc[m][:], acc[:], AF.Silu, bias=cb_t[:, m:m + 1])

        nxpool.__exit__(None, None, None)
        mamB = tc.tile_pool(name="mamB", bufs=1)
        msp = mamB.__enter__()
        ddump("zs0b", zs[0][:])
        ddump("dtr", dtr[:])
        ddump("xb0b", xbcc[0][:])
        ddump("xb8b", xbcc[8][:])


        mkt = [load(ap_, maskT[(4 + s) * 128:(5 + s) * 128, :], [128, TO], f"mkt{s}", dtype=BF16) for s in range(4)]
        cmtb = msp.tile([P, TT], BF16, tag="cmtb", name="cmtb")
        nc.vector.tensor_copy(cmtb[:], xbcc[8][P:128, :])
        hA = [msp.tile([P, P], BF16, tag=f"hA{j}", name=f"hA{j}") for j in range(16)]
        hB = [msp.tile([P, P], BF16, tag=f"hB{j}", name=f"hB{j}") for j in range(16)]
        for j in range(16):
            nc.vector.memset(hA[j][:], 0.0)
        ymam = [msp.tile([128, TO], BF16, tag=f"ymam{m}", name=f"ymam{m}") for m in range(8)]

        def trans16(src_ap, tag):
            pp = pt.tile([128, 128], F32, tag="pt", name="pt")
            nc.tensor.transpose(pp[0:128, 0:HM], src_ap, ident[0:HM, 0:HM])
            sb = mampool.tile([128, HM], F32, tag=tag, name=tag)
            nc.vector.tensor_copy(sb[:], pp[0:128, 0:HM])
            return sb

        # ---------- mamba: chunk loop ----------
        for i in range(NCH):
            sl = slice(i * LC, (i + 1) * LC)
            own = i >= 4
            hold = hA if i % 2 == 0 else hB
            hnew = hB if i % 2 == 0 else hA

            # decp[s] = exp(c_te - c[s]) * dt[s]   [HM, LC]
            decp = mampool.tile([HM, LC], F32, tag="decp", name="decp")
            nc.scalar.activation(decp[:], crow[:, sl], AF.Exp, scale=-1.0,
                                 bias=crow[:, (i + 1) * LC - 1:(i + 1) * LC])
            nc.vector.tensor_mul(decp[:], decp[:], dtr[:, sl])
            decpT = trans16(decp[:], "decpT")
            dtT = trans16(dtr[:, sl], "dtT")

            bmt_ps = pt.tile([128, 128], BF16, tag="ptb", name="ptb", bufs=1)
            nc.tensor.transpose(bmt_ps[0:128, 0:P], xbcc[8][0:P, sl], identb[0:P, 0:P])
            bmt = mampool.tile([128, P], BF16, tag="bmt", name="bmt")
            nc.vector.tensor_copy(bmt[:], bmt_ps[0:128, 0:P])
            attn_head(i)

            if own:
                # decay-in rows + transpose
                decrow = mampool.tile([HM, LC], F32, tag="decrow", name="decrow")
                nc.scalar.activation(decrow[:], crow[:, sl], AF.Exp, bias=negcte[:, i - 1:i])
                decT = trans16(decrow[:], "decT")
                ctT = trans16(crow[:, sl], "ctT")
                # M build (batched over heads)
                cflat = mampool.tile([1, HM * LC], F32, tag="cflat", name="cflat", bufs=1)
                nc.sync.dma_start(cflat[:].rearrange("o (h s) -> o h s", h=HM), crow[:, sl])
                mT = mbig.tile([128, HM * LC], F32, tag="mchain", name="mT", bufs=1)
                nc.gpsimd.partition_broadcast(mT[:], cflat[:])
                m3 = lambda ap: ap.rearrange("p (h s) -> p h s", h=HM)
                nc.vector.tensor_sub(m3(mT[:]), m3(mT[:]),
                                     ctT[:].rearrange("p (h o) -> p h o", o=1).broadcast_to([128, HM, LC]))
                nc.vector.tensor_add(m3(mT[:]), m3(mT[:]),
                                     gmask_t[:].rearrange("p (o s) -> p o s", o=1).broadcast_to([128, HM, LC]))
                mTe = mbig.tile([128, HM * LC], BF16, tag="mchainb", name="mTe", bufs=2)
                nc.scalar.activation(mTe[:], mT[:], AF.Exp)
                g_ps = pt.tile([128, 128], F32, tag="pt", name="pt")
                _mm(nc, g_ps[:], xbcc[8][0:P, sl], cmtb[:, sl], True, True)
                nc.vector.tensor_mul(m3(mTe[:]), m3(mTe[:]),
                                     g_ps[:].rearrange("p (o s) -> p o s", o=1).broadcast_to([128, HM, LC]))

            for hh in range(16):
                j, half = hh // 2, hh % 2
                rsl = slice(half * P, (half + 1) * P)
                xsT_ps = pt.tile([128, 128], BF16, tag="ptb", name="ptb", bufs=1)
                nc.tensor.transpose(xsT_ps[0:128, 0:P], xbcc[j][rsl, sl], identb[rsl, rsl])
                dtx2 = mampool.tile([128, P], BF16, tag="dtx2", name="dtx2")
                nc.vector.tensor_scalar(out=dtx2[:], in0=xsT_ps[0:128, 0:P],
                                        scalar1=decpT[:, hh:hh + 1], scalar2=None, op0=AOP.mult)
                su_ps = psu.tile([P, P], F32, tag="psu", name="psu")
                _mm(nc, su_ps[:], bmt[:], dtx2[:], True, True)
                if own:
                    dtxT = mampool.tile([128, P], BF16, tag="dtxT", name="dtxT")
                    nc.vector.tensor_scalar(out=dtxT[:], in0=xsT_ps[0:128, 0:P],
                                            scalar1=dtT[:, hh:hh + 1], scalar2=None, op0=AOP.mult)
                    y_ps = pt.tile([128, 128], F32, tag="pt", name="pt")
                    _mm(nc, y_ps[:, 0:P], mTe[:, hh * LC:(hh + 1) * LC], dtxT[:], True, True)
                    ys_ps = pt.tile([128, 128], F32, tag="pt", name="pt")
                    _mm(nc, ys_ps[:, 0:P], cmtb[:, sl], hold[hh][:], True, True)
                    ysum = mampool.tile([128, P], F32, tag="ysum", name="ysum")
                    nc.vector.tensor_scalar(out=ysum[:], in0=ys_ps[:, 0:P],
                                            scalar1=decT[:, hh:hh + 1], scalar2=None, op0=AOP.mult)
                    nc.vector.tensor_add(ysum[:], ysum[:], y_ps[:, 0:P])
                    ytr_ps = pt.tile([128, 128], F32, tag="pt", name="pt")
                    nc.tensor.transpose(ytr_ps[0:P, 0:128], ysum[:], ident[:])
                    nc.vector.scalar_tensor_tensor(
                        ymam[j][rsl, (i - 4) * LC:(i - 3) * LC], xbcc[j][rsl, sl],
                        dskip_t[rsl, j:j + 1], ytr_ps[0:P, 0:128], AOP.mult, AOP.add)
                nc.vector.scalar_tensor_tensor(hnew[hh][:], hold[hh][:],
                                               dchx16[hh][:, i:i + 1],
                                               su_ps[:], AOP.mult, AOP.add)
            if i == 3:
                for hh in range(16):
                    nc.vector.tensor_scalar(out=hnew[hh][:], in0=hnew[hh][:], scalar1=cscale_t[0:P, :],
                                            scalar2=None, op0=AOP.mult)

        # ---------- mamba: gate + RMSNorm + out proj ----------
        ddump("ymam0", ymam[0][:])
        ddump("oattn0", oattn[0][:])
        rstd_bc = scratch.tile([128, TO], F32, tag="bc512", name="rstdbc", bufs=2)
        with tc.tile_pool(name="rmsp", bufs=1) as rmsp:
            for m in range(8):
                nc.vector.tensor_mul(ymam[m][:], ymam[m][:], zs[m][:])
            ssq = pstat.tile([1, TO], F32, tag="statA", name="ssq")
            for m in range(8):
                sq2 = rmsp.tile([128, TO], F32R, tag="sq2", name="sq2", bufs=2)
                nc.scalar.activation(sq2[:], ymam[m][:], AF.Square)
                _mm(nc, ssq[:], onescol_r[:], sq2[:], m == 0, m == 7)
            r_s2 = rows.tile([1, TO], F32, tag="rsm", name="rs2", bufs=3)
            nc.scalar.activation(r_s2[:], ssq[:], AF.Sqrt, scale=1.0 / DIN, bias=epscol[0:1, :])
            r_rstd = rows.tile([1, TO], F32, tag="rsm", name="rrstd", bufs=3)
            nc.vector.reciprocal_approx_fast(r_rstd[:], r_s2[:])
            nc.gpsimd.partition_broadcast(rstd_bc[:], r_rstd[:])

            with tc.tile_pool(name="mow", bufs=1) as mow:
                wmo = [load(mow, w_mout[k * 128:(k + 1) * 128, :], [128, D], f"wmo{k}", dtype=BF16) for k in range(8)]
                om = [mxp.tile([128, TO], F32, tag=f"om{m}", name=f"om{m}") for m in range(KT)]
                for m in range(KT):
                    ps = pbig.tile([128, 512], F32, tag="mmain", name="omps")
                    for k in range(8):
                        _mm(nc, ps[:], wmo[k][:, m * 128:(m + 1) * 128], ymam[k][:], k == 0, k == 7)
                    nc.vector.tensor_mul(om[m][:], ps[:], rstd_bc[:])

        ddump("om0", om[0][:])
        mamB.__exit__(None, None, None)
        mamA.__exit__(None, None, None)
        attnpool.__exit__(None, None, None)

        # ---------- attention out-projection ----------
        oap = [mxp.tile([128, TO], F32, tag=f"oap{m}", name=f"oap{m}") for m in range(KT)]
        with tc.tile_pool(name="aow", bufs=1) as aow:
            wao_t = [load(aow, w_ao[k * 128:(k + 1) * 128, :], [128, D], f"wao{k}", dtype=BF16) for k in range(KT)]
            for m in range(KT):
                ps = pbig.tile([128, 512], F32, tag="mmain", name="mmain")
                for k in range(KT):
                    _mm(nc, ps[:], wao_t[k][:, m * 128:(m + 1) * 128], oattn[k][:], k == 0, k == KT - 1)
                nc.scalar.activation(oap[m][:], ps[:], AF.Identity, bias=bao_t[:, m:m + 1])

        # ---------- cross-attn K/V (depends only on memory; fills the gap) ----------
        chp = es.enter_context(tc.tile_pool(name="crosshold", bufs=1))
        with tc.tile_pool(name="cw_", bufs=1) as cwp:
            mt = [load(cwp, memT[k * 128:(k + 1) * 128, :], [128, TT], f"memt{k}", dtype=BF16) for k in range(KT)]
            wkv_t = [load(cwp, w_kv[k * 128:(k + 1) * 128, :], [128, 2 * D], f"wkv{k}", dtype=BF16) for k in range(KT)]
            kc = [chp.tile([128, TT], BF16, tag=f"kc{m}", name=f"kc{m}") for m in range(KT)]
            for m in range(KT):
                for g in range(2):
                    ps = pbig.tile([128, 512], F32, tag="mmain", name="mmain")
                    for k in range(KT):
                        _mm(nc, ps[:], wkv_t[k][:, m * 128:(m + 1) * 128],
                            mt[k][:, g * 512:(g + 1) * 512], k == 0, k == KT - 1)
                    nc.vector.tensor_copy(kc[m][:, g * 512:(g + 1) * 512], ps[:])
            ddump("kc0", kc[0][:])
            vcext = [chp.tile([128, NHEAD * (HD + 1)], BF16, tag=f"vcext{s}", name=f"vcext{s}") for s in range(8)]
            for s in range(8):
                ps = pbig.tile([128, 512], F32, tag="mmain", name="mmain")
                for k in range(KT):
                    _mm(nc, ps[:], mt[k][:, s * 128:(s + 1) * 128], wkv_t[k][:, D:2 * D], k == 0, k == KT - 1)
                nc.vector.tensor_copy(vcext[s][:].rearrange("p (h d) -> p h d", h=NHEAD)[:, :, 0:HD],
                                      ps[:].rearrange("p (h d) -> p h d", h=NHEAD))
                nc.vector.memset(vcext[s][:].rearrange("p (h d) -> p h d", h=NHEAD)[:, :, HD:HD + 1], 1.0)

        x1 = [persist.tile([128, TO], F32, tag=f"x1{m}", name=f"x1{m}") for m in range(KT)]
        for m in range(KT):
            nc.vector.tensor_mul(om[m][:], om[m][:], rw0bc[:])
            nc.vector.tensor_mul(oap[m][:], oap[m][:], rw1bc[:])
            nc.vector.tensor_add(x1[m][:], om[m][:], oap[m][:])
            xo = scratch.tile([128, TO], F32, tag="xo", name="xo", bufs=1)
            nc.sync.dma_start(xo[:], xT[m * 128:(m + 1) * 128, TO:TT])
            nc.vector.tensor_add(x1[m][:], x1[m][:], xo[:])
        ddump("x10", x1[0][:])

        # ---------- cross attention ----------
        x2 = [persist.tile([128, TO], F32, tag=f"x2{m}", name=f"x2{m}") for m in range(KT)]
        with tc.tile_pool(name="cattn", bufs=1) as cap, \
                tc.tile_pool(name="ppool2", bufs=9) as ppool2:
            n2 = layernorm(x1, TO, D, cap, "n2", odt=BF16)
            wq2 = [load(cap, w_q[k * 128:(k + 1) * 128, :], [128, D], f"wq2{k}", dtype=BF16) for k in range(KT)]

            qc = [cap.tile([128, TO], BF16, tag=f"qc{m}", name=f"qc{m}") for m in range(KT)]
            for m in range(KT):
                ps = pbig.tile([128, 512], F32, tag="mmain", name="mmain")
                for k in range(KT):
                    _mm(nc, ps[:], wq2[k][:, m * 128:(m + 1) * 128], n2[k][:], k == 0, k == KT - 1)
                nc.scalar.activation(qc[m][:], ps[:], AF.Identity, bias=bq_t[:, m:m + 1])
            ddump("n20", n2[0][:])
            ddump("qc0", qc[0][:])

            ocat = [cap.tile([128, TO], BF16, tag=f"ocat{m}", name=f"ocat{m}") for m in range(KT)]
            for hh in range(NHEAD):
                jj, half = hh // 2, hh % 2
                rsl = slice(half * HD, (half + 1) * HD)
                pT = []
                for s in range(8):
                    if s % 2 == 0:
                        ps = pbig.tile([128, 512], F32, tag="mmain", name="mmain")
                    else:
                        ps = pstat.tile([128, 512], F32, tag="statB", name="csps")
                    _mm(nc, ps[:], kc[jj][rsl, s * 128:(s + 1) * 128], qc[jj][rsl, :], True, True)
                    pe = ppool2.tile([128, TO], BF16, tag="pT2", name="pT2")
                    nc.scalar.activation(pe[:], ps[:], AF.Exp)
                    pT.append(pe)
                ov_ps = pbig.tile([128, 512], F32, tag="mmain", name="mmain")
                for s in range(8):
                    nc.tensor.matmul(ov_ps[0:HD + 1, :], vcext[s][:, hh * (HD + 1):(hh + 1) * (HD + 1)],
                                     pT[s][:], start=(s == 0), stop=(s == 7))
                den_sb = rows.tile([1, TO], F32, tag="rsm", name="densb2", bufs=3)
                nc.scalar.activation(den_sb[:], ov_ps[HD:HD + 1, :], AF.Copy)
                drow = rows.tile([1, TO], F32, tag="rsm", name="drow2", bufs=3)
                nc.vector.reciprocal_approx_fast(drow[:], den_sb[:])
                dbc = scratch.tile([128, TO], F32, tag="bc512", name="dbc", bufs=2)
                nc.gpsimd.partition_broadcast(dbc[:], drow[:])
                nc.vector.tensor_mul(ocat[jj][rsl, :], ov_ps[0:HD, :], dbc[rsl, :])

            ddump("ocat0", ocat[0][:])
            with tc.tile_pool(name="cow", bufs=1) as cow:
                wco_t = [load(cow, w_co[k * 128:(k + 1) * 128, :], [128, D], f"wco{k}", dtype=BF16) for k in range(KT)]
                for m in range(KT):
                    ps = pbig.tile([128, 512], F32, tag="mmain", name="mmain")
                    for k in range(KT):
                        _mm(nc, ps[:], wco_t[k][:, m * 128:(m + 1) * 128], ocat[k][:], k == 0, k == KT - 1)
                    co_sb = scratch.tile([128, TO], F32, tag="bc512", name="cosb", bufs=2)
                    nc.scalar.activation(co_sb[:], ps[:], AF.Identity, bias=bco_t[:, m:m + 1])
                    nc.vector.tensor_add(x2[m][:], x1[m][:], co_sb[:])

        ddump("x20", x2[0][:])
        # ---------- FFN ----------
        with tc.tile_pool(name="fw", bufs=1) as fw, tc.tile_pool(name="f1p", bufs=1) as f1p:
            n3 = layernorm(x2, TO, D, f1p, "n3", odt=BF16)
            ddump("n30", n3[0][:])
            wf1 = [load(fw, w_f1[k * 128:(k + 1) * 128, :], [128, DFF], f"wf1{k}", dtype=BF16) for k in range(KT)]
            wf2 = [load(fw, w_f2[k * 128:(k + 1) * 128, :], [128, D], f"wf2{k}", dtype=BF16) for k in range(16)]
            f1 = [f1p.tile([128, TO], BF16, tag=f"f1{m}", name=f"f1{m}") for m in range(16)]
            for m in range(16):
                ps = pbig.tile([128, 512], F32, tag="mmain", name="mmain")
                for k in range(KT):
                    nc.tensor.matmul(ps[:], wf1[k][:, m * 128:(m + 1) * 128], n3[k][:], start=(k == 0), stop=(k == KT - 1))
                nc.scalar.activation(f1[m][:], ps[:], AF.Gelu_apprx_tanh, bias=bf1_t[:, m:m + 1])
            ddump("f10", f1[0][:])
            for m in range(KT):
                ps = pbig.tile([128, 512], F32, tag="mmain", name="mmain")
                for k in range(16):
                    nc.tensor.matmul(ps[:], wf2[k][:, m * 128:(m + 1) * 128], f1[k][:], start=(k == 0), stop=(k == 15))
                ysb = scratch.tile([128, TO], F32, tag="bc512", name="ysb", bufs=2)
                nc.scalar.activation(ysb[:], ps[:], AF.Identity, bias=bf2_t[:, m:m + 1])
                nc.vector.tensor_add(ysb[:], ysb[:], x2[m][:])
                nc.sync.dma_start(yout[m * 128:(m + 1) * 128, :], ysb[:])

    nc.compile()
    return nc


# ================= host side =================

BF16_KEYS = {"w_inproj", "w_mout", "w_qkv", "w_router", "w_ao", "w_q",
             "w_kv", "w_co", "w_f1", "w_f2", "memT", "maskT"}


def _bf(a):
    import ml_dtypes
    return np.ascontiguousarray(np.asarray(a, dtype=np.float32).astype(ml_dtypes.bfloat16))


def _prep(inputs):
    f = lambda a: np.ascontiguousarray(np.asarray(a), dtype=np.float32)
    ip = {k: f(v) for k, v in inputs.items()}

    ln1w, ln1b = ip["ln1_w"], ip["ln1_b"]
    ln2w, ln2b = ip["ln2_w"], ip["ln2_b"]
    ln3w, ln3b = ip["ln3_w"], ip["ln3_b"]

    def pack_bias(b, ntile):
        bp = np.zeros(ntile * 128, np.float32)
        bp[:b.shape[0]] = b
        return np.ascontiguousarray(bp.reshape(ntile, 128).T)  # (128, ntile)

    w_inproj = ln1w[:, None] * ip["in_proj_w"]
    b_inproj_full = ln1b @ ip["in_proj_w"]          # (2192,)
    # cols 0..7 = z tiles, 8..16 = xBC tiles
    binp = np.zeros((128, 18), np.float32)
    binp[:, 0:8] = b_inproj_full[0:DIN].reshape(8, 128).T
    binp[:, 8:17] = b_inproj_full[DIN:DIN + CONV].reshape(9, 128).T
    dtb = (ip["dt_bias"] + b_inproj_full[DIN + CONV:]).reshape(HM, 1)

    conv_w = np.zeros((9 * 128, DCONV), np.float32)
    conv_w[:CONV] = ip["conv_w"]
    conv_w = np.ascontiguousarray(conv_w.reshape(9, 128, DCONV).transpose(1, 0, 2).reshape(128, 36))
    conv_b = pack_bias(ip["conv_b"], 9)

    negA = (-np.exp(ip["A_log"])).reshape(HM, 1)
    dskip = np.ascontiguousarray(np.repeat(ip["D_skip"], P).reshape(8, 128).T)  # (128,8)

    w_mout = ip["mamba_norm_w"][:, None] * ip["mamba_out_w"]

    scale = 1.0 / np.sqrt(HD).astype(np.float32)
    w_qkv = ln1w[:, None] * ip["attn_in_w"]
    b_qkv_full = ip["attn_in_b"] + ln1b @ ip["attn_in_w"]
    w_qkv[:, 0:D] *= scale
    b_qkv_full = b_qkv_full.copy()
    b_qkv_full[0:D] *= scale
    bqkv = pack_bias(b_qkv_full, 12)
    vbias_row = b_qkv_full[2 * D:3 * D].reshape(1, D)

    w_router = ln1w[:, None] * ip["router_w"]
    b_router = (ip["router_b"] + ln1b @ ip["router_w"]).reshape(2, 1)

    w_q = (ln2w[:, None] * ip["q_w"]) * scale
    b_q = pack_bias((ln2b @ ip["q_w"]) * scale, 4)
    w_f1 = ln3w[:, None] * ip["ffn_w1"]
    b_f1 = pack_bias(ip["ffn_b1"] + ln3b @ ip["ffn_w1"], 16)

    tri01 = np.arange(512)
    selm = np.zeros((HM, 8 * 128), np.float32)
    for j in range(8):
        for m_ in range(128):
            selm[2 * j + m_ // P, j * 128 + m_] = 1.0

    gmask = np.where(np.arange(LC)[:, None] <= np.arange(LC)[None, :], 0.0, -30000.0).astype(np.float32)

    common = dict(
        w_inproj=w_inproj, b_inproj=binp, conv_w=conv_w, conv_b=conv_b,
        dt_bias=dtb, negA=negA, dskip=dskip, w_mout=w_mout,
        w_qkv=w_qkv, b_qkv=bqkv, vbias_row=vbias_row,
        w_router=w_router, b_router=b_router,
        w_ao=ip["attn_out_w"], b_ao=pack_bias(ip["attn_out_b"], 4),
        w_q=w_q, b_q=b_q, w_kv=ip["kv_w"],
        w_co=ip["co_w"], b_co=pack_bias(ip["co_b"], 4),
        w_f1=w_f1, b_f1=b_f1, w_f2=ip["ffn_w2"], b_f2=pack_bias(ip["ffn_b2"], 4),
        gmask=gmask, selm=selm,
    )
    common = {k: (_bf(v) if k in BF16_KEYS else np.ascontiguousarray(v, dtype=np.float32))
              for k, v in common.items()}

    in_maps = []
    for c in range(8):
        b, h = c // 2, c % 2
        m = dict(common)
        xc = np.zeros((D, TT), np.float32)
        if h == 1:
            xc[:, 0:TO] = ip["x"][b, 0:TO].T
        xc[:, TO:TT] = ip["x"][b, h * TO:(h + 1) * TO].T
        m["xT"] = np.ascontiguousarray(xc)
        m["memT"] = _bf(ip["memory"][b].T)
        mk = np.zeros((TT, TO), np.float32)
        mk[TO:TT, :] = np.where(tri01[:, None] <= tri01[None, :], 0.0, -30000.0)
        m["maskT"] = _bf(mk)
        m["denc"] = np.full((1, 1), -512.0 * (1 - h), np.float32)
        m["cscale"] = np.full((128, 1), float(h), np.float32)
        in_maps.append(m)
    return in_maps


def kernel(**inputs):
    if "nc" not in _CACHE:
        _CACHE["nc"] = build_program()
    nc = _CACHE["nc"]
    in_maps = _prep(inputs)
    res = bass_utils.run_bass_kernel_spmd(nc, in_maps, core_ids=list(range(8)), trace=False)
    out = np.zeros((B_, T_, D), np.float32)
    for c in range(8):
        b, h = c // 2, c % 2
        out[b, h * TO:(h + 1) * TO, :] = res.results[c]["y"].T
    return out



# revision 93
# speedup vs baseline: 1.0086x; 1.0086x over previous
"""MoE-Mamba-Transformer block on 8 TRN2 cores (SPMD, no collectives).

Sharding: core c -> (batch b = c//2, sequence-half h = c%2); each core
computes the output for its 512 "own" tokens given full-sequence context,
host gathers. Device is feature-major ([feature, token]); host transposes
x/memory/output, folds LayerNorm affine params into downstream weights,
and pre-rounds all GEMM weights to bf16 (PE runs 1 cycle/row vs 4 for
fp32; bf16 weights also re-enable fast weight load).

Schedule: router + self-attention projections are emitted before the
Mamba section, and self-attention head i is emitted after Mamba chunk i
inside the 8-chunk SSD loop, so PE/ACT fill the DVE-bound chunk loop's
gaps (engine queues are FIFO per engine; program order = issue order).
Cross-attention K/V (memory-only deps) fill the post-loop gap. Softmax
prefix masking is folded into a per-core denominator-correction bias;
only diagonal key tiles carry an additive causal mask. Norm/softmax
reciprocals use the single-instruction DVE approx (SBUF-staged - the
approx misreads PSUM operands). The Mamba scan uses the chunked SSD
formulation (matmuls + one cumsum in fp32; per-head small matmuls in
bf16 with fp32 PSUM accumulation; bf16 inter-chunk state).
"""

import numpy as np
from contextlib import ExitStack

import concourse.bass as bass
import concourse.mybir as mybir
import concourse.tile as tile
from concourse import bacc, bass_utils
from concourse.masks import make_identity

F32 = mybir.dt.float32
F32R = mybir.dt.float32r
BF16 = mybir.dt.bfloat16
AOP = mybir.AluOpType
AF = mybir.ActivationFunctionType

D = 512
NHEAD = 8
HD = 64
DIN = 1024
HM = 16
P = 64
NST = 64
DCONV = 4
CONV = 1152
DPROJ = 2192
DFF = 2048
EPS = 1e-5
TO = 512
TT = 1024
LC = 128
NCH = TT // LC
KT = D // 128
B_, T_, S_ = 4, 1024, 1024

_CACHE = {}


def _mm(nc, out, lhsT, rhs, start, stop):
    nc.tensor.matmul(out, lhsT, rhs, start=start, stop=stop)


def _mmr(nc, out, lhsT, rhs, start, stop):
    nc.tensor.matmul(out, lhsT.bitcast(F32R), rhs.bitcast(F32R), start=start, stop=stop)


def build_program(dbg=False):
    nc = bacc.Bacc("TRN2", target_bir_lowering=False, debug=False, num_devices=8)
    din = lambda name, shape, dt=F32: nc.dram_tensor(name, shape, dt, kind="ExternalInput").ap()
    _dbg_outs = {}

    def dout(name, shape, dt=F32):
        _dbg_outs[name] = nc.dram_tensor("dbg_" + name, shape, dt, kind="ExternalOutput").ap()
        return _dbg_outs[name]

    xT = din("xT", [D, TT])
    memT = din("memT", [D, TT], BF16)
    maskT = din("maskT", [TT, TO], BF16)
    gmask = din("gmask", [LC, LC])
    selm = din("selm", [HM, 8 * 128])
    cscale = din("cscale", [128, 1])
    denc = din("denc", [1, 1])
    w_inproj = din("w_inproj", [D, DPROJ], BF16)
    b_inproj = din("b_inproj", [128, 18])
    conv_w = din("conv_w", [128, 36])
    conv_b = din("conv_b", [128, 9])
    dt_bias = din("dt_bias", [HM, 1])
    negA = din("negA", [HM, 1])
    dskip = din("dskip", [128, 8])
    w_mout = din("w_mout", [DIN, D], BF16)
    w_qkv = din("w_qkv", [D, 3 * D], BF16)
    b_qkv = din("b_qkv", [128, 12])
    vbias_row = din("vbias_row", [1, D])
    w_router = din("w_router", [D, 2], BF16)
    b_router = din("b_router", [2, 1])
    w_ao = din("w_ao", [D, D], BF16)
    b_ao = din("b_ao", [128, 4])
    w_q = din("w_q", [D, D], BF16)
    b_q = din("b_q", [128, 4])
    w_kv = din("w_kv", [D, 2 * D], BF16)
    w_co = din("w_co", [D, D], BF16)
    b_co = din("b_co", [128, 4])
    w_f1 = din("w_f1", [D, DFF], BF16)
    b_f1 = din("b_f1", [128, 16])
    w_f2 = din("w_f2", [DFF, D], BF16)
    b_f2 = din("b_f2", [128, 4])
    yout = nc.dram_tensor("y", [D, TO], F32, kind="ExternalOutput").ap()
    if dbg:
        for nm, sh in [("dtr", [HM, TT]), ("crow", [HM, TT]), ("om0", [128, TO]),
                       ("x10", [128, TO]), ("x20", [128, TO])]:
            dout(nm, sh)
        for nm, sh in [("zs0b", [128, TO]), ("xb0b", [128, TT]), ("xb8b", [128, TT]),
                       ("nx", [128, TT]), ("ymam0", [128, TO]), ("ymam0n", [128, TO]),
                       ("qf0", [128, TO]), ("kf0", [128, TT]), ("oattn0", [128, TO]),
                       ("n20", [128, TO]), ("qc0", [128, TO]), ("kc0", [128, TT]),
                       ("ocat0", [128, TO]), ("n30", [128, TO]), ("f10", [128, TO])]:
            dout(nm, sh, BF16)

    es = ExitStack()
    with es:
        tc = es.enter_context(tile.TileContext(nc))
        persist = es.enter_context(tc.tile_pool(name="persist", bufs=1))
        pbig = es.enter_context(tc.tile_pool(name="pbig", bufs=2, space="PSUM"))
        pt = es.enter_context(tc.tile_pool(name="pt", bufs=2, space="PSUM"))
        psu = es.enter_context(tc.tile_pool(name="psu", bufs=1, space="PSUM"))
        pstat = es.enter_context(tc.tile_pool(name="pstat", bufs=1, space="PSUM"))
        scratch = es.enter_context(tc.tile_pool(name="scratch", bufs=1))
        rows = es.enter_context(tc.tile_pool(name="rows", bufs=1))
        mampool = es.enter_context(tc.tile_pool(name="mampool", bufs=2))
        mbig = es.enter_context(tc.tile_pool(name="mbig", bufs=1))

        ident = persist.tile([128, 128], F32, tag="ident", name="ident")
        make_identity(nc, ident[:])
        identb = persist.tile([128, 128], BF16, tag="identb", name="identb")
        make_identity(nc, identb[:])
        onescol = persist.tile([128, 1], F32, tag="onescol", name="onescol")
        nc.vector.memset(onescol[:], 1.0)
        onescol_r = persist.tile([128, 1], F32R, tag="onescolr", name="onescolr")
        nc.vector.tensor_copy(onescol_r[:], onescol[:])
        epscol = persist.tile([128, 1], F32, tag="epscol", name="epscol")
        nc.vector.memset(epscol[:], EPS)

        def load(pool, ap, shape, tag, dtype=F32):
            t = pool.tile(shape, dtype, tag=tag)
            if dtype == F32R:
                nc.gpsimd.dma_start(t[:], ap)
            else:
                nc.sync.dma_start(t[:], ap)
            return t

        cscale_t = load(persist, cscale[:], [128, 1], "cscale")
        denc_t = load(persist, denc[:], [1, 1], "denc")
        dtb_t = load(persist, dt_bias[:], [HM, 1], "dtb")
        negA_t = load(persist, negA[:], [HM, 1], "negA")
        dskip_t = load(persist, dskip[:], [128, 8], "dskip")
        gmask_t = load(persist, gmask[:], [LC, LC], "gmask")
        binp_t = load(persist, b_inproj[:], [128, 18], "binp")
        cw_t = load(persist, conv_w[:], [128, 36], "cw")
        cb_t = load(persist, conv_b[:], [128, 9], "cb")
        bqkv_t = load(persist, b_qkv[:], [128, 12], "bqkv")
        bao_t = load(persist, b_ao[:], [128, 4], "bao")
        bq_t = load(persist, b_q[:], [128, 4], "bq")
        bco_t = load(persist, b_co[:], [128, 4], "bco")
        bf1_t = load(persist, b_f1[:], [128, 16], "bf1")
        bf2_t = load(persist, b_f2[:], [128, 4], "bf2")
        brt_t = load(persist, b_router[:], [2, 1], "brt")
        vbr_t = load(persist, vbias_row[:], [1, D], "vbr")


        def ddump(name, ap, bf=False):
            if not dbg:
                return
            if ap.dtype == F32R:
                ap = ap.bitcast(F32)
            nc.sync.dma_start(_dbg_outs[name], ap)

        mxp = es.enter_context(tc.tile_pool(name="mixpool", bufs=1))
        ppool = es.enter_context(tc.tile_pool(name="ppool", bufs=9))
        attnpool = tc.tile_pool(name="attnp", bufs=1)
        ap_ = attnpool.__enter__()
        mamA = tc.tile_pool(name="mamA", bufs=1)
        mspA = mamA.__enter__()
        nxpool = tc.tile_pool(name="nxpool", bufs=1)
        nxp = nxpool.__enter__()
        xtpool = tc.tile_pool(name="xtpool", bufs=1)
        xtp = xtpool.__enter__()
        xt = [load(xtp, xT[k * 128:(k + 1) * 128, :], [128, TT], f"xt{k}") for k in range(KT)]

        # ---------- feature-major LayerNorm ----------
        def layernorm(xtiles, TL, nfeat, outpool, tag, odt=F32):
            nk = len(xtiles)
            bca = scratch.tile([128, TL], F32, tag="lnbc", name="lnbca", bufs=2)
            bcb = scratch.tile([128, TL], F32, tag="lnbc", name="lnbcb", bufs=2)
            onescol_b = scratch.tile([128, 1], BF16, tag="onescolb", name="onescolb", bufs=1)
            nc.vector.tensor_copy(onescol_b[:], onescol[:])
            for g in range(TL // 512):
                sl = slice(g * 512, (g + 1) * 512)
                sum_ps = pstat.tile([1, 512], F32, tag="statA", name="statA")
                sq_ps = pstat.tile([1, 512], F32, tag="statB", name="statB")
                for k in range(nk):
                    xb = scratch.tile([128, 512], BF16, tag="lnxb", name="lnxb", bufs=2)
                    nc.vector.tensor_copy(xb[:], xtiles[k][:, sl])
                    _mm(nc, sum_ps[:], onescol_b[:], xb[:], k == 0, k == nk - 1)
                    sqt = scratch.tile([128, 512], BF16, tag="lnsq", name="lnsq", bufs=2)
                    nc.scalar.activation(sqt[:], xb[:], AF.Square)
                    _mm(nc, sq_ps[:], onescol_b[:], sqt[:], k == 0, k == nk - 1)
                r_m2 = rows.tile([1, 512], F32, tag="rsm", name="r1", bufs=3)
                nc.scalar.activation(r_m2[:], sum_ps[:], AF.Square, scale=1.0 / nfeat)
                r_v = rows.tile([1, 512], F32, tag="rsm", name="r2", bufs=3)
                nc.vector.scalar_tensor_tensor(r_v[:], sq_ps[:], 1.0 / nfeat, r_m2[:], AOP.mult, AOP.subtract)
                r_s = rows.tile([1, 512], F32, tag="rsm", name="r3", bufs=3)
                nc.scalar.activation(r_s[:], r_v[:], AF.Sqrt, bias=epscol[0:1, :])
                ra = rows.tile([1, 512], F32, tag="rsm", name="ra", bufs=3)
                nc.vector.reciprocal_approx_fast(ra[:], r_s[:])
                r_mean = rows.tile([1, 512], F32, tag="rsm", name="r5", bufs=3)
                nc.scalar.activation(r_mean[:], sum_ps[:], AF.Copy, scale=1.0 / nfeat)
                rb = rows.tile([1, 512], F32, tag="rsm", name="rb", bufs=3)
                nc.vector.scalar_tensor_tensor(rb[:], r_mean[:], -1.0, ra[:], AOP.mult, AOP.mult)
                nc.gpsimd.partition_broadcast(bca[:, sl], ra[:])
                nc.gpsimd.partition_broadcast(bcb[:, sl], rb[:])
            out = []
            for k in range(nk):
                if odt != F32:
                    o = scratch.tile([128, TL], F32, tag="lnstage", name="lnstage", bufs=1)
                    orr = outpool.tile([128, TL], odt, tag=f"{tag}{k}", name=f"{tag}{k}")
                    for g in range(TL // 512):
                        sl2 = slice(g * 512, (g + 1) * 512)
                        nc.vector.tensor_mul(o[:, sl2], xtiles[k][:, sl2], bca[:, sl2])
                        nc.vector.tensor_add(orr[:, sl2], o[:, sl2], bcb[:, sl2])
                    o = orr
                else:
                    o = outpool.tile([128, TL], F32, tag=f"{tag}{k}", name=f"{tag}{k}")
                    nc.vector.tensor_mul(o[:], xtiles[k][:], bca[:])
                    nc.vector.tensor_add(o[:], o[:], bcb[:])
                out.append(o)
            return out

        nx = layernorm(xt, TT, D, nxp, "nx", odt=BF16)
        ddump("nx", nx[0][:])
        xtpool.__exit__(None, None, None)

        # ---------- router (early; needs only nx) ----------
        rps = pstat.tile([2, TO], F32, tag="statA", name="statB")
        with tc.tile_pool(name="rw", bufs=1) as rwp:
            wr = [load(rwp, w_router[k * 128:(k + 1) * 128, :], [128, 2], f"wr{k}", dtype=BF16) for k in range(KT)]
            for k in range(KT):
                _mm(nc, rps[:], wr[k][:], nx[k][:, TO:TT], k == 0, k == KT - 1)
            er = rows.tile([2, TO], F32R, tag="rer", name="er", bufs=1)
            nc.scalar.activation(er[:], rps[:], AF.Exp, bias=brt_t[:])
            dsum_ps = pstat.tile([1, TO], F32, tag="statB", name="dsumps")
            _mm(nc, dsum_ps[:], onescol_r[0:2, :], er[:], True, True)
            den_sb = rows.tile([1, TO], F32, tag="rsm", name="densb3", bufs=3)
            nc.scalar.activation(den_sb[:], dsum_ps[:], AF.Copy)
            drec = rows.tile([1, TO], F32, tag="rsm", name="drec", bufs=3)
            nc.vector.reciprocal_approx_fast(drec[:], den_sb[:])
            rw0r = rows.tile([1, TO], F32, tag="rsm", name="rw0r", bufs=3)
            nc.vector.tensor_mul(rw0r[:], er[0:1, :], drec[:])
            rw1r = rows.tile([1, TO], F32, tag="rsm", name="rw1r", bufs=3)
            nc.vector.tensor_scalar(out=rw1r[:], in0=rw0r[:], scalar1=-1.0, scalar2=1.0, op0=AOP.mult, op1=AOP.add)
            rw0b_r = rows.tile([1, TO], BF16, tag="rsmb", name="rw0br", bufs=2)
            nc.vector.tensor_copy(rw0b_r[:], rw0r[:])
            rw1b_r = rows.tile([1, TO], BF16, tag="rsmb", name="rw1br", bufs=2)
            nc.vector.tensor_copy(rw1b_r[:], rw1r[:])
            rw0bc = mxp.tile([128, TO], BF16, tag="rw0bc", name="rw0bc")
            rw1bc = mxp.tile([128, TO], BF16, tag="rw1bc", name="rw1bc")
            nc.gpsimd.partition_broadcast(rw0bc[:], rw0b_r[:])
            nc.gpsimd.partition_broadcast(rw1bc[:], rw1b_r[:])

        # ---------- attention projections (early; need only nx) ----------
        oattn = [mxp.tile([128, TO], BF16, tag=f"oattn{m}", name=f"oattn{m}") for m in range(KT)]
        with tc.tile_pool(name="aw", bufs=1) as aw:
            wq = [load(aw, w_qkv[k * 128:(k + 1) * 128, :], [128, 3 * D], f"wqkv{k}", dtype=BF16) for k in range(KT)]
            kf = [ap_.tile([128, TT], BF16, tag=f"kf{m}", name=f"kf{m}") for m in range(KT)]
            for m in range(KT):
                for g in range(2):
                    ps = pbig.tile([128, 512], F32, tag="mmain", name="mmain")
                    for k in range(KT):
                        _mm(nc, ps[:], wq[k][:, D + m * 128:D + (m + 1) * 128],
                            nx[k][:, g * 512:(g + 1) * 512], k == 0, k == KT - 1)
                    nc.scalar.activation(kf[m][:, g * 512:(g + 1) * 512], ps[:], AF.Identity,
                                         bias=bqkv_t[:, 4 + m:5 + m])
            qf = [ap_.tile([128, TO], BF16, tag=f"qf{m}", name=f"qf{m}") for m in range(KT)]
            for m in range(KT):
                ps = pbig.tile([128, 512], F32, tag="mmain", name="mmain")
                for k in range(KT):
                    _mm(nc, ps[:], wq[k][:, m * 128:(m + 1) * 128], nx[k][:, TO:TT], k == 0, k == KT - 1)
                nc.scalar.activation(qf[m][:], ps[:], AF.Identity, bias=bqkv_t[:, m:m + 1])
            ddump("qf0", qf[0][:])
            ddump("kf0", kf[0][:])
            vext = [ap_.tile([128, NHEAD * (HD + 1)], BF16, tag=f"vext{s}", name=f"vext{s}") for s in range(8)]
            vb_bc = aw.tile([128, D], F32, tag="vbbc", name="vbbc")
            nc.gpsimd.partition_broadcast(vb_bc[:], vbr_t[:])
            for s in range(8):
                ps = pbig.tile([128, 512], F32, tag="mmain", name="mmain")
                for k in range(KT):
                    _mm(nc, ps[:], nx[k][:, s * 128:(s + 1) * 128], wq[k][:, 2 * D:3 * D], k == 0, k == KT - 1)
                v3 = lambda apx: apx.rearrange("p (h d) -> p h d", h=NHEAD)
                nc.vector.tensor_add(vext[s][:].rearrange("p (h d) -> p h d", h=NHEAD)[:, :, 0:HD],
                                     v3(ps[:]), v3(vb_bc[:]))
                nc.vector.memset(vext[s][:].rearrange("p (h d) -> p h d", h=NHEAD)[:, :, HD:HD + 1], 1.0)

        def attn_head(hh):
            jj, half = hh // 2, hh % 2
            rsl = slice(half * HD, (half + 1) * HD)
            pT = []
            for s in range(8):
                ps = pbig.tile([128, 512], F32, tag="mmain", name="mmain")
                _mm(nc, ps[:], kf[jj][rsl, s * 128:(s + 1) * 128], qf[jj][rsl, :], True, True)
                pe = ppool.tile([128, TO], BF16, tag="pT", name="pT")
                if s < 4:
                    nc.scalar.activation(pe[:], ps[:], AF.Exp)
                else:
                    nc.vector.tensor_add(pe[:], ps[:], mkt[s - 4][:])
                    nc.scalar.activation(pe[:], pe[:], AF.Exp)
                pT.append(pe)
            ov_ps = pbig.tile([128, 512], F32, tag="mmain", name="mmain")
            for s in range(8):
                nc.tensor.matmul(ov_ps[0:HD + 1, :], vext[s][:, hh * (HD + 1):(hh + 1) * (HD + 1)],
                                 pT[s][:], start=(s == 0), stop=(s == 7))
            den_sb = rows.tile([1, TO], F32, tag="rsm", name="densb", bufs=3)
            nc.scalar.activation(den_sb[:], ov_ps[HD:HD + 1, :], AF.Identity, bias=denc_t[0:1, :])
            drow = rows.tile([1, TO], F32, tag="rsm", name="drow", bufs=3)
            nc.vector.reciprocal_approx_fast(drow[:], den_sb[:])
            dbc = scratch.tile([128, TO], F32, tag="bc512", name="dbc", bufs=2)
            nc.gpsimd.partition_broadcast(dbc[:], drow[:])
            nc.vector.tensor_mul(oattn[jj][rsl, :], ov_ps[0:HD, :], dbc[rsl, :])

        # ---------- mamba: in_proj + depthwise conv + dt ----------
        xbcc = [mspA.tile([128, TT], BF16, tag=f"xbcc{m}", name=f"xbcc{m}") for m in range(9)]
        zs = [mspA.tile([128, TO], BF16, tag=f"zs{m}", name=f"zs{m}") for m in range(8)]
        dtr = mspA.tile([HM, TT], F32, tag="dtr", name="dtr")

        with tc.tile_pool(name="mwz", bufs=1) as mwz:
            wdt = [load(mwz, w_inproj[k * 128:(k + 1) * 128, DPROJ - HM:DPROJ], [128, HM], f"wdt{k}", dtype=BF16) for k in range(KT)]
            for g in range(2):
                ps = pstat.tile([HM, 512], F32, tag="statA", name="dtps")
                for k in range(KT):
                    _mm(nc, ps[:], wdt[k][:], nx[k][:, g * 512:(g + 1) * 512], k == 0, k == KT - 1)
                dsl = dtr[:, g * 512:(g + 1) * 512]
                nc.scalar.activation(dsl, ps[:], AF.Sigmoid, bias=dtb_t[:])
                nc.scalar.activation(dsl, dsl, AF.Ln)
                nc.vector.scalar_tensor_tensor(dsl, ps[:], dtb_t[:], dsl, AOP.add, AOP.subtract)
            for mh in range(2):
                wz = [load(mwz, w_inproj[k * 128:(k + 1) * 128, mh * 512:(mh + 1) * 512],
                           [128, 512], f"wz{k}", dtype=BF16) for k in range(KT)]
                for mm_ in range(4):
                    m = mh * 4 + mm_
                    ps = pbig.tile([128, 512], F32, tag="mmain", name="zps")
                    for k in range(KT):
                        _mm(nc, ps[:], wz[k][:, mm_ * 128:(mm_ + 1) * 128], nx[k][:, TO:TT], k == 0, k == KT - 1)
                    nc.scalar.activation(zs[m][:], ps[:], AF.Silu, bias=binp_t[:, m:m + 1])

        # ---------- mamba: cumsum + chunk decays ----------
        logdA = mspA.tile([HM, TT], F32, tag="logdA", name="logdA")
        nc.vector.tensor_scalar(out=logdA[:], in0=dtr[:], scalar1=negA_t[:], scalar2=None, op0=AOP.mult)
        zrow = mspA.tile([HM, TT], F32, tag="zrow", name="zrow")
        nc.vector.memset(zrow[:], 0.0)
        crow = mspA.tile([HM, TT], F32, tag="crow", name="crow")
        nc.vector.tensor_tensor_scan(crow[:], logdA[:], zrow[:], 0.0, AOP.add, AOP.add)

        negcte = mspA.tile([HM, NCH], F32, tag="negcte", name="negcte")
        dchunk_all = mspA.tile([HM, NCH], F32, tag="dchunk", name="dchunk")
        for i in range(NCH):
            te = (i + 1) * LC - 1
            nc.vector.tensor_scalar(out=negcte[:, i:i + 1], in0=crow[:, te:te + 1],
                                    scalar1=-1.0, scalar2=None, op0=AOP.mult)
        for i in range(NCH):
            te = (i + 1) * LC - 1
            if i == 0:
                nc.scalar.activation(dchunk_all[:, 0:1], crow[:, te:te + 1], AF.Exp)
            else:
                nc.scalar.activation(dchunk_all[:, i:i + 1], crow[:, te:te + 1], AF.Exp,
                                     bias=negcte[:, i - 1:i])

        ddump("crow", crow[:])
        selm_t = load(mspA, selm[:], [HM, 8 * 128], "selm")
        dchx16 = []
        for j in range(8):
            pp = pstat.tile([128, P], F32, tag="statB", name="psu")
            _mm(nc, pp[:, 0:8], selm_t[:, j * 128:(j + 1) * 128], dchunk_all[:], True, True)
            for half in range(2):
                sb = mspA.tile([P, 8], F32, tag=f"dchx{2 * j + half}", name=f"dchx{2 * j + half}")
                nc.vector.tensor_copy(sb[:], pp[half * P:(half + 1) * P, 0:8])
                dchx16.append(sb)

        with tc.tile_pool(name="mwx", bufs=1) as mwx, tc.tile_pool(name="rawp", bufs=2) as rawp:
            for wh in range(3):
                wx = [load(mwx, w_inproj[k * 128:(k + 1) * 128, DIN + wh * 384:DIN + (wh + 1) * 384],
                           [128, 384], f"wx{k}", dtype=BF16) for k in range(KT)]
                for mm_ in range(3):
                    m = wh * 3 + mm_
                    raw = rawp.tile([128, DCONV - 1 + TT], BF16, tag="raw", name="raw", bufs=2)
                    nc.vector.memset(raw[:, 0:DCONV - 1], 0.0)
                    for g in range(2):
                        ps = pbig.tile([128, 512], F32, tag="mmain", name="xps")
                        for k in range(KT):
                            _mm(nc, ps[:], wx[k][:, mm_ * 128:(mm_ + 1) * 128],
                                nx[k][:, g * 512:(g + 1) * 512], k == 0, k == KT - 1)
                        nc.scalar.activation(raw[:, 3 + g * 512:3 + (g + 1) * 512], ps[:],
                                             AF.Identity, bias=binp_t[:, 8 + m:9 + m])
                    acc = xbcc[m]
                    nc.vector.tensor_scalar(out=acc[:], in0=raw[:, 0:TT],
                                            scalar1=cw_t[:, 4 * m:4 * m + 1], scalar2=None, op0=AOP.mult)
                    for kk in range(1, DCONV):
                        nc.vector.scalar_tensor_tensor(acc[:], raw[:, kk:kk + TT],
                                                       cw_t[:, 4 * m + kk:4 * m + kk + 1], acc[:],
                                                       AOP.mult, AOP.add)
                    nc.scalar.activation(xbcc[m][:], acc[:], AF.Silu, bias=cb_t[:, m:m + 1])

        nxpool.__exit__(None, None, None)
        mamB = tc.tile_pool(name="mamB", bufs=1)
        msp = mamB.__enter__()
        ddump("zs0b", zs[0][:])
        ddump("dtr", dtr[:])
        ddump("xb0b", xbcc[0][:])
        ddump("xb8b", xbcc[8][:])


        mkt = [load(ap_, maskT[(4 + s) * 128:(5 + s) * 128, :], [128, TO], f"mkt{s}", dtype=BF16) for s in range(4)]
        cmtb = msp.tile([P, TT], BF16, tag="cmtb", name="cmtb")
        nc.vector.tensor_copy(cmtb[:], xbcc[8][P:128, :])
        hA = [msp.tile([P, P], BF16, tag=f"hA{j}", name=f"hA{j}") for j in range(16)]
        hB = [msp.tile([P, P], BF16, tag=f"hB{j}", name=f"hB{j}") for j in range(16)]
        for j in range(16):
            nc.vector.memset(hA[j][:], 0.0)
        ymam = [msp.tile([128, TO], BF16, tag=f"ymam{m}", name=f"ymam{m}") for m in range(8)]

        def trans16(src_ap, tag):
            pp = pt.tile([128, 128], F32, tag="pt", name="pt")
            nc.tensor.transpose(pp[0:128, 0:HM], src_ap, ident[0:HM, 0:HM])
            sb = mampool.tile([128, HM], F32, tag=tag, name=tag)
            nc.vector.tensor_copy(sb[:], pp[0:128, 0:HM])
            return sb

        # ---------- mamba: chunk loop ----------
        for i in range(NCH):
            sl = slice(i * LC, (i + 1) * LC)
            own = i >= 4
            hold = hA if i % 2 == 0 else hB
            hnew = hB if i % 2 == 0 else hA

            # decp[s] = exp(c_te - c[s]) * dt[s]   [HM, LC]
            decp = mampool.tile([HM, LC], F32, tag="decp", name="decp")
            nc.scalar.activation(decp[:], crow[:, sl], AF.Exp, scale=-1.0,
                                 bias=crow[:, (i + 1) * LC - 1:(i + 1) * LC])
            nc.vector.tensor_mul(decp[:], decp[:], dtr[:, sl])
            decpT = trans16(decp[:], "decpT")
            dtT = trans16(dtr[:, sl], "dtT")

            bmt_ps = pt.tile([128, 128], BF16, tag="ptb", name="ptb", bufs=1)
            nc.tensor.transpose(bmt_ps[0:128, 0:P], xbcc[8][0:P, sl], identb[0:P, 0:P])
            bmt = mampool.tile([128, P], BF16, tag="bmt", name="bmt")
            nc.vector.tensor_copy(bmt[:], bmt_ps[0:128, 0:P])
            attn_head(i)

            if own:
                # decay-in rows + transpose
                decrow = mampool.tile([HM, LC], F32, tag="decrow", name="decrow")
                nc.scalar.activation(decrow[:], crow[:, sl], AF.Exp, bias=negcte[:, i - 1:i])
                decT = trans16(decrow[:], "decT")
                ctT = trans16(crow[:, sl], "ctT")
                # M build (batched over heads)
                cflat = mampool.tile([1, HM * LC], F32, tag="cflat", name="cflat", bufs=1)
                nc.sync.dma_start(cflat[:].rearrange("o (h s) -> o h s", h=HM), crow[:, sl])
                mT = mbig.tile([128, HM * LC], F32, tag="mchain", name="mT", bufs=1)
                nc.gpsimd.partition_broadcast(mT[:], cflat[:])
                m3 = lambda ap: ap.rearrange("p (h s) -> p h s", h=HM)
                nc.vector.tensor_sub(m3(mT[:]), m3(mT[:]),
                                     ctT[:].rearrange("p (h o) -> p h o", o=1).broadcast_to([128, HM, LC]))
                nc.vector.tensor_add(m3(mT[:]), m3(mT[:]),
                                     gmask_t[:].rearrange("p (o s) -> p o s", o=1).broadcast_to([128, HM, LC]))
                mTe = mbig.tile([128, HM * LC], BF16, tag="mchainb", name="mTe", bufs=2)
                nc.scalar.activation(mTe[:], mT[:], AF.Exp)
                g_ps = pt.tile([128, 128], F32, tag="pt", name="pt")
                _mm(nc, g_ps[:], xbcc[8][0:P, sl], cmtb[:, sl], True, True)
                nc.vector.tensor_mul(m3(mTe[:]), m3(mTe[:]),
                                     g_ps[:].rearrange("p (o s) -> p o s", o=1).broadcast_to([128, HM, LC]))

            for hh in range(16):
                j, half = hh // 2, hh % 2
                rsl = slice(half * P, (half + 1) * P)
                xsT_ps = pt.tile([128, 128], BF16, tag="ptb", name="ptb", bufs=1)
                nc.tensor.transpose(xsT_ps[0:128, 0:P], xbcc[j][rsl, sl], identb[rsl, rsl])
                dtx2 = mampool.tile([128, P], BF16, tag="dtx2", name="dtx2")
                nc.vector.tensor_scalar(out=dtx2[:], in0=xsT_ps[0:128, 0:P],
                                        scalar1=decpT[:, hh:hh + 1], scalar2=None, op0=AOP.mult)
                su_ps = psu.tile([P, P], F32, tag="psu", name="psu")
                _mm(nc, su_ps[:], bmt[:], dtx2[:], True, True)
                if own:
                    dtxT = mampool.tile([128, P], BF16, tag="dtxT", name="dtxT")
                    nc.vector.tensor_scalar(out=dtxT[:], in0=xsT_ps[0:128, 0:P],
                                            scalar1=dtT[:, hh:hh + 1], scalar2=None, op0=AOP.mult)
                    y_ps = pt.tile([128, 128], F32, tag="pt", name="pt")
                    _mm(nc, y_ps[:, 0:P], mTe[:, hh * LC:(hh + 1) * LC], dtxT[:], True, True)
                    ys_ps = pt.tile([128, 128], F32, tag="pt", name="pt")
                    _mm(nc, ys_ps[:, 0:P], cmtb[:, sl], hold[hh][:], True, True)
                    ysum = mampool.tile([128, P], F32, tag="ysum", name="ysum")
                    nc.vector.tensor_scalar(out=ysum[:], in0=ys_ps[:, 0:P],
                                            scalar1=decT[:, hh:hh + 1], scalar2=None, op0=AOP.mult)
                    nc.vector.tensor_add(ysum[:], ysum[:], y_ps[:, 0:P])
                    ytr_ps = pt.tile([128, 128], F32, tag="pt", name="pt")
                    nc.tensor.transpose(ytr_ps[0:P, 0:128], ysum[:], ident[:])
                    nc.vector.scalar_tensor_tensor(
                        ymam[j][rsl, (i - 4) * LC:(i - 3) * LC], xbcc[j][rsl, sl],
                        dskip_t[rsl, j:j + 1], ytr_ps[0:P, 0:128], AOP.mult, AOP.add)
                nc.vector.scalar_tensor_tensor(hnew[hh][:], hold[hh][:],
                                               dchx16[hh][:, i:i + 1],
                                               su_ps[:], AOP.mult, AOP.add)
            if i == 3:
                for hh in range(16):
                    nc.vector.tensor_scalar(out=hnew[hh][:], in0=hnew[hh][:], scalar1=cscale_t[0:P, :],
                                            scalar2=None, op0=AOP.mult)

        # ---------- mamba: gate + RMSNorm + out proj ----------
        ddump("ymam0", ymam[0][:])
        ddump("oattn0", oattn[0][:])
        rstd_bc = scratch.tile([128, TO], F32, tag="bc512", name="rstdbc", bufs=2)
        with tc.tile_pool(name="rmsp", bufs=1) as rmsp:
            for m in range(8):
                nc.vector.tensor_mul(ymam[m][:], ymam[m][:], zs[m][:])
            ssq = pstat.tile([1, TO], F32, tag="statA", name="ssq")
            for m in range(8):
                sq2 = rmsp.tile([128, TO], F32R, tag="sq2", name="sq2", bufs=2)
                nc.scalar.activation(sq2[:], ymam[m][:], AF.Square)
                _mm(nc, ssq[:], onescol_r[:], sq2[:], m == 0, m == 7)
            r_s2 = rows.tile([1, TO], F32, tag="rsm", name="rs2", bufs=3)
            nc.scalar.activation(r_s2[:], ssq[:], AF.Sqrt, scale=1.0 / DIN, bias=epscol[0:1, :])
            r_rstd = rows.tile([1, TO], F32, tag="rsm", name="rrstd", bufs=3)
            nc.vector.reciprocal_approx_fast(r_rstd[:], r_s2[:])
            nc.gpsimd.partition_broadcast(rstd_bc[:], r_rstd[:])

            with tc.tile_pool(name="mow", bufs=1) as mow:
                wmo = [load(mow, w_mout[k * 128:(k + 1) * 128, :], [128, D], f"wmo{k}", dtype=BF16) for k in range(8)]
                om = [mxp.tile([128, TO], F32, tag=f"om{m}", name=f"om{m}") for m in range(KT)]
                for m in range(KT):
                    ps = pbig.tile([128, 512], F32, tag="mmain", name="omps")
                    for k in range(8):
                        _mm(nc, ps[:], wmo[k][:, m * 128:(m + 1) * 128], ymam[k][:], k == 0, k == 7)
                    nc.vector.tensor_mul(om[m][:], ps[:], rstd_bc[:])

        ddump("om0", om[0][:])
        mamB.__exit__(None, None, None)
        mamA.__exit__(None, None, None)
        attnpool.__exit__(None, None, None)

        # ---------- attention out-projection ----------
        oap = [mxp.tile([128, TO], F32, tag=f"oap{m}", name=f"oap{m}") for m in range(KT)]
        with tc.tile_pool(name="aow", bufs=1) as aow:
            wao_t = [load(aow, w_ao[k * 128:(k + 1) * 128, :], [128, D], f"wao{k}", dtype=BF16) for k in range(KT)]
            for m in range(KT):
                ps = pbig.tile([128, 512], F32, tag="mmain", name="mmain")
                for k in range(KT):
                    _mm(nc, ps[:], wao_t[k][:, m * 128:(m + 1) * 128], oattn[k][:], k == 0, k == KT - 1)
                nc.scalar.activation(oap[m][:], ps[:], AF.Identity, bias=bao_t[:, m:m + 1])

        # ---------- cross-attn K/V (depends only on memory; fills the gap) ----------
        chp = es.enter_context(tc.tile_pool(name="crosshold", bufs=1))
        with tc.tile_pool(name="cw_", bufs=1) as cwp:
            mt = [load(cwp, memT[k * 128:(k + 1) * 128, :], [128, TT], f"memt{k}", dtype=BF16) for k in range(KT)]
            wkv_t = [load(cwp, w_kv[k * 128:(k + 1) * 128, :], [128, 2 * D], f"wkv{k}", dtype=BF16) for k in range(KT)]
            kc = [chp.tile([128, TT], BF16, tag=f"kc{m}", name=f"kc{m}") for m in range(KT)]
            for m in range(KT):
                for g in range(2):
                    ps = pbig.tile([128, 512], F32, tag="mmain", name="mmain")
                    for k in range(KT):
                        _mm(nc, ps[:], wkv_t[k][:, m * 128:(m + 1) * 128],
                            mt[k][:, g * 512:(g + 1) * 512], k == 0, k == KT - 1)
                    nc.vector.tensor_copy(kc[m][:, g * 512:(g + 1) * 512], ps[:])
            ddump("kc0", kc[0][:])
            vcext = [chp.tile([128, NHEAD * (HD + 1)], BF16, tag=f"vcext{s}", name=f"vcext{s}") for s in range(8)]
            for s in range(8):
                ps = pbig.tile([128, 512], F32, tag="mmain", name="mmain")
                for k in range(KT):
                    _mm(nc, ps[:], mt[k][:, s * 128:(s + 1) * 128], wkv_t[k][:, D:2 * D], k == 0, k == KT - 1)
                nc.vector.tensor_copy(vcext[s][:].rearrange("p (h d) -> p h d", h=NHEAD)[:, :, 0:HD],
                                      ps[:].rearrange("p (h d) -> p h d", h=NHEAD))
                nc.vector.memset(vcext[s][:].rearrange("p (h d) -> p h d", h=NHEAD)[:, :, HD:HD + 1], 1.0)

        x1 = [persist.tile([128, TO], F32, tag=f"x1{m}", name=f"x1{m}") for m in range(KT)]
        for m in range(KT):
            nc.vector.tensor_mul(om[m][:], om[m][:], rw0bc[:])
            nc.vector.tensor_mul(oap[m][:], oap[m][:], rw1bc[:])
            nc.vector.tensor_add(x1[m][:], om[m][:], oap[m][:])
            xo = scratch.tile([128, TO], F32, tag="xo", name="xo", bufs=1)
            nc.sync.dma_start(xo[:], xT[m * 128:(m + 1) * 128, TO:TT])
            nc.vector.tensor_add(x1[m][:], x1[m][:], xo[:])
        ddump("x10", x1[0][:])

        # ---------- cross attention ----------
        x2 = [persist.tile([128, TO], F32, tag=f"x2{m}", name=f"x2{m}") for m in range(KT)]
        with tc.tile_pool(name="cattn", bufs=1) as cap, \
                tc.tile_pool(name="ppool2", bufs=9) as ppool2:
            n2 = layernorm(x1, TO, D, cap, "n2", odt=BF16)
            wq2 = [load(cap, w_q[k * 128:(k + 1) * 128, :], [128, D], f"wq2{k}", dtype=BF16) for k in range(KT)]

            qc = [cap.tile([128, TO], BF16, tag=f"qc{m}", name=f"qc{m}") for m in range(KT)]
            for m in range(KT):
                ps = pbig.tile([128, 512], F32, tag="mmain", name="mmain")
                for k in range(KT):
                    _mm(nc, ps[:], wq2[k][:, m * 128:(m + 1) * 128], n2[k][:], k == 0, k == KT - 1)
                nc.scalar.activation(qc[m][:], ps[:], AF.Identity, bias=bq_t[:, m:m + 1])
            ddump("n20", n2[0][:])
            ddump("qc0", qc[0][:])

            ocat = [cap.tile([128, TO], BF16, tag=f"ocat{m}", name=f"ocat{m}") for m in range(KT)]
            for hh in range(NHEAD):
                jj, half = hh // 2, hh % 2
                rsl = slice(half * HD, (half + 1) * HD)
                pT = []
                for s in range(8):
                    if s % 2 == 0:
                        ps = pbig.tile([128, 512], F32, tag="mmain", name="mmain")
                    else:
                        ps = pstat.tile([128, 512], F32, tag="statB", name="csps")
                    _mm(nc, ps[:], kc[jj][rsl, s * 128:(s + 1) * 128], qc[jj][rsl, :], True, True)
                    pe = ppool2.tile([128, TO], BF16, tag="pT2", name="pT2")
                    nc.scalar.activation(pe[:], ps[:], AF.Exp)
                    pT.append(pe)
                ov_ps = pbig.tile([128, 512], F32, tag="mmain", name="mmain")
                for s in range(8):
                    nc.tensor.matmul(ov_ps[0:HD + 1, :], vcext[s][:, hh * (HD + 1):(hh + 1) * (HD + 1)],
                                     pT[s][:], start=(s == 0), stop=(s == 7))
                den_sb = rows.tile([1, TO], F32, tag="rsm", name="densb2", bufs=3)
                nc.scalar.activation(den_sb[:], ov_ps[HD:HD + 1, :], AF.Copy)
                drow = rows.tile([1, TO], F32, tag="rsm", name="drow2", bufs=3)
                nc.vector.reciprocal_approx_fast(drow[:], den_sb[:])
                dbc = scratch.tile([128, TO], F32, tag="bc512", name="dbc", bufs=2)
                nc.gpsimd.partition_broadcast(dbc[:], drow[:])
                nc.vector.tensor_mul(ocat[jj][rsl, :], ov_ps[0:HD, :], dbc[rsl, :])

            ddump("ocat0", ocat[0][:])
            with tc.tile_pool(name="cow", bufs=1) as cow:
                wco_t = [load(cow, w_co[k * 128:(k + 1) * 128, :], [128, D], f"wco{k}", dtype=BF16) for k in range(KT)]
                for m in range(KT):
                    ps = pbig.tile([128, 512], F32, tag="mmain", name="mmain")
                    for k in range(KT):
                        _mm(nc, ps[:], wco_t[k][:, m * 128:(m + 1) * 128], ocat[k][:], k == 0, k == KT - 1)
                    co_sb = scratch.tile([128, TO], F32, tag="bc512", name="cosb", bufs=2)
                    nc.scalar.activation(co_sb[:], ps[:], AF.Identity, bias=bco_t[:, m:m + 1])
                    nc.vector.tensor_add(x2[m][:], x1[m][:], co_sb[:])

        ddump("x20", x2[0][:])
        # ---------- FFN ----------
        with tc.tile_pool(name="fw", bufs=1) as fw, tc.tile_pool(name="f1p", bufs=1) as f1p:
            n3 = layernorm(x2, TO, D, f1p, "n3", odt=BF16)
            ddump("n30", n3[0][:])
            wf1 = [load(fw, w_f1[k * 128:(k + 1) * 128, :], [128, DFF], f"wf1{k}", dtype=BF16) for k in range(KT)]
            wf2 = [load(fw, w_f2[k * 128:(k + 1) * 128, :], [128, D], f"wf2{k}", dtype=BF16) for k in range(16)]
            f1 = [f1p.tile([128, TO], BF16, tag=f"f1{m}", name=f"f1{m}") for m in range(16)]
            for m in range(16):
                ps = pbig.tile([128, 512], F32, tag="mmain", name="mmain")
                for k in range(KT):
                    nc.tensor.matmul(ps[:], wf1[k][:, m * 128:(m + 1) * 128], n3[k][:], start=(k == 0), stop=(k == KT - 1))
                nc.scalar.activation(f1[m][:], ps[:], AF.Gelu_apprx_tanh, bias=bf1_t[:, m:m + 1])
            ddump("f10", f1[0][:])
            for m in range(KT):
                ps = pbig.tile([128, 512], F32, tag="mmain", name="mmain")
                for k in range(16):
                    nc.tensor.matmul(ps[:], wf2[k][:, m * 128:(m + 1) * 128], f1[k][:], start=(k == 0), stop=(k == 15))
                ysb = scratch.tile([128, TO], F32, tag="bc512", name="ysb", bufs=2)
                nc.scalar.activation(ysb[:], ps[:], AF.Identity, bias=bf2_t[:, m:m + 1])
                nc.vector.tensor_add(ysb[:], ysb[:], x2[m][:])
                nc.sync.dma_start(yout[m * 128:(m + 1) * 128, :], ysb[:])

    nc.compile()
    return nc


# ================= host side =================

BF16_KEYS = {"w_inproj", "w_mout", "w_qkv", "w_router", "w_ao", "w_q",
             "w_kv", "w_co", "w_f1", "w_f2", "memT", "maskT"}


def _bf(a):
    import ml_dtypes
    return np.ascontiguousarray(np.asarray(a, dtype=np.float32).astype(ml_dtypes.bfloat16))


def _prep(inputs):
    f = lambda a: np.ascontiguousarray(np.asarray(a), dtype=np.float32)
    ip = {k: f(v) for k, v in inputs.items()}

    ln1w, ln1b = ip["ln1_w"], ip["ln1_b"]
    ln2w, ln2b = ip["ln2_w"], ip["ln2_b"]
    ln3w, ln3b = ip["ln3_w"], ip["ln3_b"]

    def pack_bias(b, ntile):
        bp = np.zeros(ntile * 128, np.float32)
        bp[:b.shape[0]] = b
        return np.ascontiguousarray(bp.reshape(ntile, 128).T)  # (128, ntile)

    w_inproj = ln1w[:, None] * ip["in_proj_w"]
    b_inproj_full = ln1b @ ip["in_proj_w"]          # (2192,)
    # cols 0..7 = z tiles, 8..16 = xBC tiles
    binp = np.zeros((128, 18), np.float32)
    binp[:, 0:8] = b_inproj_full[0:DIN].reshape(8, 128).T
    binp[:, 8:17] = b_inproj_full[DIN:DIN + CONV].reshape(9, 128).T
    dtb = (ip["dt_bias"] + b_inproj_full[DIN + CONV:]).reshape(HM, 1)

    conv_w = np.zeros((9 * 128, DCONV), np.float32)
    conv_w[:CONV] = ip["conv_w"]
    conv_w = np.ascontiguousarray(conv_w.reshape(9, 128, DCONV).transpose(1, 0, 2).reshape(128, 36))
    conv_b = pack_bias(ip["conv_b"], 9)

    negA = (-np.exp(ip["A_log"])).reshape(HM, 1)
    dskip = np.ascontiguousarray(np.repeat(ip["D_skip"], P).reshape(8, 128).T)  # (128,8)

    w_mout = ip["mamba_norm_w"][:, None] * ip["mamba_out_w"]

    scale = 1.0 / np.sqrt(HD).astype(np.float32)
    w_qkv = ln1w[:, None] * ip["attn_in_w"]
    b_qkv_full = ip["attn_in_b"] + ln1b @ ip["attn_in_w"]
    w_qkv[:, 0:D] *= scale
    b_qkv_full = b_qkv_full.copy()
    b_qkv_full[0:D] *= scale
    bqkv = pack_bias(b_qkv_full, 12)
    vbias_row = b_qkv_full[2 * D:3 * D].reshape(1, D)

    w_router = ln1w[:, None] * ip["router_w"]
    b_router = (ip["router_b"] + ln1b @ ip["router_w"]).reshape(2, 1)

    w_q = (ln2w[:, None] * ip["q_w"]) * scale
    b_q = pack_bias((ln2b @ ip["q_w"]) * scale, 4)
    w_f1 = ln3w[:, None] * ip["ffn_w1"]
    b_f1 = pack_bias(ip["ffn_b1"] + ln3b @ ip["ffn_w1"], 16)

    tri01 = np.arange(512)
    selm = np.zeros((HM, 8 * 128), np.float32)
    for j in range(8):
        for m_ in range(128):
            selm[2 * j + m_ // P, j * 128 + m_] = 1.0

    gmask = np.where(np.arange(LC)[:, None] <= np.arange(LC)[None, :], 0.0, -30000.0).astype(np.float32)

    common = dict(
        w_inproj=w_inproj, b_inproj=binp, conv_w=conv_w, conv_b=conv_b,
        dt_bias=dtb, negA=negA, dskip=dskip, w_mout=w_mout,
        w_qkv=w_qkv, b_qkv=bqkv, vbias_row=vbias_row,
        w_router=w_router, b_router=b_router,
        w_ao=ip["attn_out_w"], b_ao=pack_bias(ip["attn_out_b"], 4),
        w_q=w_q, b_q=b_q, w_kv=ip["kv_w"],
        w_co=ip["co_w"], b_co=pack_bias(ip["co_b"], 4),
        w_f1=w_f1, b_f1=b_f1, w_f2=ip["ffn_w2"], b_f2=pack_bias(ip["ffn_b2"], 4),
        gmask=gmask, selm=selm,
    )
    common = {k: (_bf(v) if k in BF16_KEYS else np.ascontiguousarray(v, dtype=np.float32))
              for k, v in common.items()}

    in_maps = []
    for c in range(8):
        b, h = c // 2, c % 2
        m = dict(common)
        xc = np.zeros((D, TT), np.float32)
        if h == 1:
            xc[:, 0:TO] = ip["x"][b, 0:TO].T
        xc[:, TO:TT] = ip["x"][b, h * TO:(h + 1) * TO].T
        m["xT"] = np.ascontiguousarray(xc)
        m["memT"] = _bf(ip["memory"][b].T)
        mk = np.zeros((TT, TO), np.float32)
        mk[TO:TT, :] = np.where(tri01[:, None] <= tri01[None, :], 0.0, -30000.0)
        m["maskT"] = _bf(mk)
        m["denc"] = np.full((1, 1), -512.0 * (1 - h), np.float32)
        m["cscale"] = np.full((128, 1), float(h), np.float32)
        in_maps.append(m)
    return in_maps


def kernel(**inputs):
    if "nc" not in _CACHE:
        _CACHE["nc"] = build_program()
    nc = _CACHE["nc"]
    in_maps = _prep(inputs)
    res = bass_utils.run_bass_kernel_spmd(nc, in_maps, core_ids=list(range(8)), trace=False)
    out = np.zeros((B_, T_, D), np.float32)
    for c in range(8):
        b, h = c // 2, c % 2
        out[b, h * TO:(h + 1) * TO, :] = res.results[c]["y"].T
    return out

